# revision 5
# baseline (speedup 1.0000x reference)
"""Trainium2 Bass kernel for nn_Dihedral: per-dihedral torsion energies.

V[m] = k0*(1-cos(theta-a0)) + k1*(1-cos(2*theta-a1)),  theta = dihedral angle
of atoms mapping[0..3, m].

Math (atan2/cos-free): with b1=p1-p0, b2=p2-p1, b3=p3-p2, n1=b1xb2, n2=b2xb3,
x = n1.n2, u = (n1xn2).b2, L = |b2|, X = x*L, H^2 = X^2+u^2:
  cos(theta) = X/H, sin(theta) = -u/H
  V = A - (B*X + C*u)/H - (D*(X^2-u^2) + E*X*u)/H^2
with host-precomputed per-type-tuple tables
  A = k0+k1, B = k0*cos(a0), C = -k0*sin(a0), D = k1*cos(a1), E = -2*k1*sin(a1).

Sharding: dihedral dim M=4M split evenly over 8 cores (data parallel);
atoms (pos+type packed, [2M,4] f32) and the packed param table ([26^4,8] f32)
replicated on every core. No collectives.

Device work per 128-dihedral slot column: 4 vector-indirect DMA gathers of
packed atom rows + 1 gather of the param row (param index computed on-device
from the gathered type lanes), then DVE/ACT elementwise math.
"""
import numpy as np

import concourse.bacc as bacc
import concourse.bass as bass
import concourse.tile as tile
from concourse import mybir
from concourse.bass import IndirectOffsetOnAxis
from concourse.bass_utils import run_bass_kernel_spmd

N_ATOMS = 2_000_000
N_DIH = 4_000_000
N_TYPES = 26
N_CORES = 8
P = 128

MC = N_DIH // N_CORES          # dihedrals per core
UNROLL = 16                    # slot columns per loop iteration
CHUNK = P * UNROLL             # dihedrals per iteration
NITER = -(-MC // CHUNK)        # loop iterations
MC_PAD = NITER * CHUNK         # padded per-core dihedral count

F32 = mybir.dt.float32
I32 = mybir.dt.int32


def _build_nc():
    nc = bacc.Bacc("TRN2", target_bir_lowering=False, debug=False)
    atoms = nc.dram_tensor("atoms", [N_ATOMS, 4], F32, kind="ExternalInput")
    prm = nc.dram_tensor("prm", [N_TYPES ** 4, 8], F32, kind="ExternalInput")
    mp = nc.dram_tensor("mp", [4, MC_PAD], I32, kind="ExternalInput")
    vout = nc.dram_tensor("V", [MC_PAD], F32, kind="ExternalOutput")

    with tile.TileContext(nc) as tc:
        import contextlib

        with contextlib.ExitStack() as ctx:
            pool = ctx.enter_context(tc.tile_pool(name="work", bufs=2))
            tmp = ctx.enter_context(tc.tile_pool(name="tmp", bufs=2))
            gp = ctx.enter_context(tc.tile_pool(name="gp", bufs=2))

            with tc.For_i(0, MC_PAD, CHUNK) as it:
                # ---- load this iteration's mapping columns -> [128, 4*UNROLL] i32
                idx_t = pool.tile([P, 4 * UNROLL], I32, tag="idx", name="idx_t")
                for r in range(4):
                    nc.sync.dma_start(
                        out=idx_t[:, r * UNROLL : (r + 1) * UNROLL],
                        in_=mp[r, bass.ds(it, CHUNK)].rearrange(
                            "(p j) -> p j", p=P
                        ),
                    )

                # ---- gather packed atom rows for the 4 roles.
                # Each gather writes its OWN tile: slice-disjoint writes to a
                # shared tile serialize in Tile's dep tracking and expose the
                # full ~73us indirect-DMA latency; independent tiles keep
                # ~4*UNROLL gathers in flight (~4.75us/inst effective).
                ga = [[None] * UNROLL for _ in range(4)]
                for j in range(UNROLL):
                    for r in range(4):
                        g = gp.tile([P, 4], F32, tag=f"g{r}_{j}", name=f"g{r}_{j}")
                        nc.gpsimd.indirect_dma_start(
                            out=g[:],
                            out_offset=None,
                            in_=atoms[:],
                            in_offset=IndirectOffsetOnAxis(
                                ap=idx_t[:, r * UNROLL + j : r * UNROLL + j + 1],
                                axis=0,
                            ),
                        )
                        ga[r][j] = g
                pr = [pool.tile([P, UNROLL, 4], F32, tag=f"p{r}", name=f"p{r}") for r in range(4)]
                for j in range(UNROLL):
                    for r in range(4):
                        nc.vector.tensor_copy(pr[r][:, j, :], ga[r][j][:])

                def lane(r, c):
                    # component c (0..3) of role r as a [P, UNROLL] strided AP
                    return pr[r][:, :, c]

                def t_new(tag):
                    return tmp.tile([P, UNROLL], F32, tag=tag, name=tag)

                tt = nc.vector.tensor_tensor
                OP = mybir.AluOpType

                # ---- param index from type lanes (exact in f32: < 2^24)
                s1 = t_new("s1")
                nc.vector.tensor_scalar_mul(s1, lane(1, 3), float(N_TYPES ** 2))
                s2 = t_new("s2")
                nc.vector.scalar_tensor_tensor(
                    s2, lane(0, 3), float(N_TYPES ** 3), s1, OP.mult, OP.add
                )
                s3 = t_new("s3")
                nc.vector.scalar_tensor_tensor(
                    s3, lane(2, 3), float(N_TYPES), lane(3, 3), OP.mult, OP.add
                )
                pidx_f = t_new("pidx_f")
                tt(pidx_f, s2, s3, OP.add)
                pidx = tmp.tile([P, UNROLL], I32, tag="pidx", name="pidx")
                nc.vector.tensor_copy(pidx, pidx_f)

                # ---- gather param rows [A,B,C,D,E,...] (own tile per gather)
                gv = []
                for j in range(UNROLL):
                    g = gp.tile([P, 8], F32, tag=f"gv{j}", name=f"gv{j}")
                    nc.gpsimd.indirect_dma_start(
                        out=g[:],
                        out_offset=None,
                        in_=prm[:],
                        in_offset=IndirectOffsetOnAxis(
                            ap=pidx[:, j : j + 1], axis=0
                        ),
                    )
                    gv.append(g)
                pv = pool.tile([P, UNROLL, 8], F32, tag="pv", name="pv")
                for j in range(UNROLL):
                    nc.vector.tensor_copy(pv[:, j, :], gv[j][:])

                # ---- geometry: b1, b2, b3 (SoA, unit-stride outputs)
                b = {}
                for (name, hi, lo) in (("b1", 1, 0), ("b2", 2, 1), ("b3", 3, 2)):
                    for c in range(3):
                        dst = t_new(f"{name}{c}")
                        tt(dst, lane(hi, c), lane(lo, c), OP.subtract)
                        b[f"{name}{c}"] = dst

                def cross(pre, ax, ay, az, bx, by, bz):
                    out = []
                    for (cc, (m1, m2, m3, m4)) in enumerate(
                        ((ay, bz, az, by), (az, bx, ax, bz), (ax, by, ay, bx))
                    ):
                        u1 = t_new(f"{pre}u{cc}")
                        tt(u1, m1, m2, OP.mult)
                        u2 = t_new(f"{pre}v{cc}")
                        tt(u2, m3, m4, OP.mult)
                        o = t_new(f"{pre}{cc}")
                        tt(o, u1, u2, OP.subtract)
                        out.append(o)
                    return out

                n1 = cross("n1", b["b10"], b["b11"], b["b12"],
                           b["b20"], b["b21"], b["b22"])
                n2 = cross("n2", b["b20"], b["b21"], b["b22"],
                           b["b30"], b["b31"], b["b32"])

                def dot(pre, a3, c3):
                    d1 = t_new(f"{pre}d1")
                    tt(d1, a3[0], c3[0], OP.mult)
                    d2 = t_new(f"{pre}d2")
                    tt(d2, a3[1], c3[1], OP.mult)
                    d3 = t_new(f"{pre}d3")
                    tt(d3, a3[2], c3[2], OP.mult)
                    s = t_new(f"{pre}s")
                    tt(s, d1, d2, OP.add)
                    o = t_new(f"{pre}")
                    tt(o, s, d3, OP.add)
                    return o

                x = dot("x", n1, n2)
                t3 = cross("t3", *n1, *n2)
                b2v = [b["b20"], b["b21"], b["b22"]]
                u = dot("u", t3, b2v)
                l2 = dot("l2", b2v, b2v)
                lv = t_new("lv")
                nc.scalar.sqrt(lv, l2)
                xx0 = t_new("xx0")
                tt(xx0, x, lv, OP.mult)         # X = x*L
                # +eps: degenerate dihedrals (repeated atoms -> H=0) resolve
                # to cos(theta)=1, matching the reference's atan2(0,0)=0.
                xx = t_new("xx")
                nc.vector.tensor_scalar_add(xx, xx0, 1e-15)

                x2 = t_new("x2")
                tt(x2, xx, xx, OP.mult)
                u2t = t_new("u2t")
                tt(u2t, u, u, OP.mult)
                h2 = t_new("h2")
                tt(h2, x2, u2t, OP.add)
                rh2 = t_new("rh2")
                nc.vector.reciprocal(rh2, h2)    # 1/H^2
                rh = t_new("rh")
                nc.scalar.sqrt(rh, rh2)          # 1/H

                # P1 = B*X + C*u
                p1a = t_new("p1a")
                tt(p1a, pv[:, :, 1], xx, OP.mult)
                p1b = t_new("p1b")
                tt(p1b, pv[:, :, 2], u, OP.mult)
                p1 = t_new("p1")
                tt(p1, p1a, p1b, OP.add)
                # Q = D*(X^2-u^2) + E*X*u
                xmu = t_new("xmu")
                tt(xmu, x2, u2t, OP.subtract)
                q1 = t_new("q1")
                tt(q1, pv[:, :, 3], xmu, OP.mult)
                xu = t_new("xu")
                tt(xu, xx, u, OP.mult)
                q2 = t_new("q2")
                tt(q2, pv[:, :, 4], xu, OP.mult)
                qq = t_new("qq")
                tt(qq, q1, q2, OP.add)
                # V = A - P1/H - Q/H^2
                m1t = t_new("m1t")
                tt(m1t, p1, rh, OP.mult)
                m2t = t_new("m2t")
                tt(m2t, qq, rh2, OP.mult)
                v1 = t_new("v1")
                tt(v1, pv[:, :, 0], m1t, OP.subtract)
                v2 = t_new("v2")
                tt(v2, v1, m2t, OP.subtract)
                # reference NaNs exactly when |b2|==0 (its 0/0 normalize);
                # 0*recip(L^2) is 0 normally, NaN when L^2==0.
                rl2 = t_new("rl2")
                nc.vector.reciprocal(rl2, l2)
                nanz = t_new("nanz")
                nc.vector.tensor_scalar_mul(nanz, rl2, 0.0)
                vt = pool.tile([P, UNROLL], F32, tag="vt", name="vt")
                tt(vt, v2, nanz, OP.add)

                nc.sync.dma_start(
                    out=vout[bass.ds(it, CHUNK)].rearrange("(p j) -> p j", p=P),
                    in_=vt[:],
                )

    nc.finalize()
    return nc


_NC_CACHE = None


def _get_nc():
    global _NC_CACHE
    if _NC_CACHE is None:
        _NC_CACHE = _build_nc()
    return _NC_CACHE


def kernel(pos, theta_0, k_0, theta_1, k_1, mapping, atom_types):
    pos = np.asarray(pos, dtype=np.float32)
    mapping = np.asarray(mapping)
    atom_types = np.asarray(atom_types)

    # pack atoms: [x, y, z, type] per row (type exact as f32)
    atoms = np.empty((N_ATOMS, 4), np.float32)
    atoms[:, :3] = pos
    atoms[:, 3] = atom_types.astype(np.float32)

    # pack parameter tables -> [26^4, 8] f32 rows [A, B, C, D, E, 0, 0, 0]
    a0 = np.asarray(theta_0, np.float32).reshape(-1)
    a1 = np.asarray(theta_1, np.float32).reshape(-1)
    K0 = np.asarray(k_0, np.float32).reshape(-1)
    K1 = np.asarray(k_1, np.float32).reshape(-1)
    prm = np.zeros((N_TYPES ** 4, 8), np.float32)
    prm[:, 0] = K0 + K1
    prm[:, 1] = K0 * np.cos(a0)
    prm[:, 2] = -K0 * np.sin(a0)
    prm[:, 3] = K1 * np.cos(a1)
    prm[:, 4] = -2.0 * K1 * np.sin(a1)

    mp32 = np.ascontiguousarray(mapping.astype(np.int32))

    nc = _get_nc()
    in_maps = []
    for c in range(N_CORES):
        sl = mp32[:, c * MC : (c + 1) * MC]
        mp_pad = np.empty((4, MC_PAD), np.int32)
        mp_pad[:, :MC] = sl
        # padding uses distinct atoms so no 0/0 NaNs are generated
        mp_pad[:, MC:] = np.array([[0], [1], [2], [3]], np.int32)
        in_maps.append({"atoms": atoms, "prm": prm, "mp": mp_pad})

    res = run_bass_kernel_spmd(nc, in_maps, core_ids=list(range(N_CORES)))
    out = np.empty(N_DIH, np.float32)
    for c in range(N_CORES):
        out[c * MC : (c + 1) * MC] = res.results[c]["V"][:MC]
    return out


# revision 6
# speedup vs baseline: 1.4357x; 1.4357x over previous
"""Trainium2 Bass kernel for nn_Dihedral: per-dihedral torsion energies.

V[m] = k0*(1-cos(theta-a0)) + k1*(1-cos(2*theta-a1)),  theta = dihedral angle
of atoms mapping[0..3, m].

Math (atan2/cos-free): with b1=p1-p0, b2=p2-p1, b3=p3-p2, n1=b1xb2, n2=b2xb3,
x = n1.n2, u = (n1xn2).b2, L = |b2|, X = x*L, H^2 = X^2+u^2:
  cos(theta) = X/H, sin(theta) = -u/H
  V = A - (B*X + C*u)/H - (D*(X^2-u^2) + E*X*u)/H^2
with host-precomputed per-type-tuple tables
  A = k0+k1, B = k0*cos(a0), C = -k0*sin(a0), D = k1*cos(a1), E = -2*k1*sin(a1).

Sharding: dihedral dim M=4M split evenly over 8 cores (data parallel);
atoms (pos+type packed, [2M,4] f32) and the packed param table ([26^4,8] f32)
replicated on every core. No collectives.

Device work per 128-dihedral slot column: 4 vector-indirect DMA gathers of
packed atom rows + 1 gather of the param row (param index computed on-device
from the gathered type lanes), then DVE/ACT elementwise math.
"""
import numpy as np

import concourse.bacc as bacc
import concourse.bass as bass
import concourse.tile as tile
from concourse import mybir
from concourse.bass import IndirectOffsetOnAxis
from concourse.bass_utils import run_bass_kernel_spmd

N_ATOMS = 2_000_000
N_DIH = 4_000_000
N_TYPES = 26
N_CORES = 8
P = 128

MC = N_DIH // N_CORES          # dihedrals per core
UNROLL = 16                    # slot columns per loop iteration
CHUNK = P * UNROLL             # dihedrals per iteration
NITER = -(-MC // CHUNK)        # loop iterations
MC_PAD = NITER * CHUNK         # padded per-core dihedral count

F32 = mybir.dt.float32
I32 = mybir.dt.int32


def _build_nc():
    nc = bacc.Bacc("TRN2", target_bir_lowering=False, debug=False)
    pos3 = nc.dram_tensor("pos3", [N_ATOMS, 3], F32, kind="ExternalInput")
    prm = nc.dram_tensor("prm", [N_TYPES ** 4, 5], F32, kind="ExternalInput")
    mp = nc.dram_tensor("mp", [4, MC_PAD], I32, kind="ExternalInput")
    vout = nc.dram_tensor("V", [MC_PAD], F32, kind="ExternalOutput")

    with tile.TileContext(nc) as tc:
        import contextlib

        with contextlib.ExitStack() as ctx:
            pool = ctx.enter_context(tc.tile_pool(name="work", bufs=2))
            tmp = ctx.enter_context(tc.tile_pool(name="tmp", bufs=2))
            gp = ctx.enter_context(tc.tile_pool(name="gp", bufs=2))

            with tc.For_i(0, MC_PAD, CHUNK) as it:
                # ---- load this iteration's mapping columns -> [128, 4*UNROLL] i32
                idx_t = pool.tile([P, 4 * UNROLL], I32, tag="idx", name="idx_t")
                for r in range(4):
                    nc.sync.dma_start(
                        out=idx_t[:, r * UNROLL : (r + 1) * UNROLL],
                        in_=mp[r, bass.ds(it, CHUNK)].rearrange(
                            "(p j) -> p j", p=P
                        ),
                    )
                # mapping entries carry the atom type in bits 21+ (atom ids
                # fit in 21 bits); unpack: aidx = idx & 0x1FFFFF, ty = idx>>21
                OPb = mybir.AluOpType
                aidx = pool.tile([P, 4 * UNROLL], I32, tag="aidx", name="aidx")
                nc.vector.tensor_scalar(aidx, idx_t, 0x1FFFFF, None,
                                        OPb.bitwise_and)
                tysh = tmp.tile([P, 4 * UNROLL], I32, tag="tysh", name="tysh")
                nc.vector.tensor_scalar(tysh, idx_t, 21, None,
                                        OPb.logical_shift_right)
                tyf = pool.tile([P, 4 * UNROLL], F32, tag="tyf", name="tyf")
                nc.vector.tensor_copy(tyf, tysh)

                # ---- gather packed atom rows for the 4 roles.
                # Each gather writes its OWN tile: slice-disjoint writes to a
                # shared tile serialize in Tile's dep tracking and expose the
                # full ~73us indirect-DMA latency; independent tiles keep
                # ~4*UNROLL gathers in flight (~4.75us/inst effective).
                ga = [[None] * UNROLL for _ in range(4)]
                for j in range(UNROLL):
                    for r in range(4):
                        g = gp.tile([P, 3], F32, tag=f"g{r}_{j}", name=f"g{r}_{j}")
                        nc.gpsimd.indirect_dma_start(
                            out=g[:],
                            out_offset=None,
                            in_=pos3[:],
                            in_offset=IndirectOffsetOnAxis(
                                ap=aidx[:, r * UNROLL + j : r * UNROLL + j + 1],
                                axis=0,
                            ),
                        )
                        ga[r][j] = g
                pr = [pool.tile([P, UNROLL, 3], F32, tag=f"p{r}", name=f"p{r}") for r in range(4)]
                for j in range(UNROLL):
                    for r in range(4):
                        nc.vector.tensor_copy(pr[r][:, j, :], ga[r][j][:])

                def lane(r, c):
                    # component c (0..3) of role r as a [P, UNROLL] strided AP
                    return pr[r][:, :, c]

                def t_new(tag):
                    return tmp.tile([P, UNROLL], F32, tag=tag, name=tag)

                tt = nc.vector.tensor_tensor
                OP = mybir.AluOpType

                # ---- param index from type lanes (exact in f32: < 2^24)
                def tcol(r):
                    return tyf[:, r * UNROLL : (r + 1) * UNROLL]

                s1 = t_new("s1")
                nc.vector.tensor_scalar_mul(s1, tcol(1), float(N_TYPES ** 2))
                s2 = t_new("s2")
                nc.vector.scalar_tensor_tensor(
                    s2, tcol(0), float(N_TYPES ** 3), s1, OP.mult, OP.add
                )
                s3 = t_new("s3")
                nc.vector.scalar_tensor_tensor(
                    s3, tcol(2), float(N_TYPES), tcol(3), OP.mult, OP.add
                )
                pidx_f = t_new("pidx_f")
                tt(pidx_f, s2, s3, OP.add)
                pidx = tmp.tile([P, UNROLL], I32, tag="pidx", name="pidx")
                nc.vector.tensor_copy(pidx, pidx_f)

                # ---- gather param rows [A,B,C,D,E,...] (own tile per gather)
                gv = []
                for j in range(UNROLL):
                    g = gp.tile([P, 5], F32, tag=f"gv{j}", name=f"gv{j}")
                    nc.gpsimd.indirect_dma_start(
                        out=g[:],
                        out_offset=None,
                        in_=prm[:],
                        in_offset=IndirectOffsetOnAxis(
                            ap=pidx[:, j : j + 1], axis=0
                        ),
                    )
                    gv.append(g)
                pv = pool.tile([P, UNROLL, 5], F32, tag="pv", name="pv")
                for j in range(UNROLL):
                    nc.vector.tensor_copy(pv[:, j, :], gv[j][:])

                # ---- geometry: b1, b2, b3 (SoA, unit-stride outputs)
                b = {}
                for (name, hi, lo) in (("b1", 1, 0), ("b2", 2, 1), ("b3", 3, 2)):
                    for c in range(3):
                        dst = t_new(f"{name}{c}")
                        tt(dst, lane(hi, c), lane(lo, c), OP.subtract)
                        b[f"{name}{c}"] = dst

                def cross(pre, ax, ay, az, bx, by, bz):
                    out = []
                    for (cc, (m1, m2, m3, m4)) in enumerate(
                        ((ay, bz, az, by), (az, bx, ax, bz), (ax, by, ay, bx))
                    ):
                        u1 = t_new(f"{pre}u{cc}")
                        tt(u1, m1, m2, OP.mult)
                        u2 = t_new(f"{pre}v{cc}")
                        tt(u2, m3, m4, OP.mult)
                        o = t_new(f"{pre}{cc}")
                        tt(o, u1, u2, OP.subtract)
                        out.append(o)
                    return out

                n1 = cross("n1", b["b10"], b["b11"], b["b12"],
                           b["b20"], b["b21"], b["b22"])
                n2 = cross("n2", b["b20"], b["b21"], b["b22"],
                           b["b30"], b["b31"], b["b32"])

                def dot(pre, a3, c3):
                    d1 = t_new(f"{pre}d1")
                    tt(d1, a3[0], c3[0], OP.mult)
                    d2 = t_new(f"{pre}d2")
                    tt(d2, a3[1], c3[1], OP.mult)
                    d3 = t_new(f"{pre}d3")
                    tt(d3, a3[2], c3[2], OP.mult)
                    s = t_new(f"{pre}s")
                    tt(s, d1, d2, OP.add)
                    o = t_new(f"{pre}")
                    tt(o, s, d3, OP.add)
                    return o

                x = dot("x", n1, n2)
                t3 = cross("t3", *n1, *n2)
                b2v = [b["b20"], b["b21"], b["b22"]]
                u = dot("u", t3, b2v)
                l2 = dot("l2", b2v, b2v)
                lv = t_new("lv")
                nc.scalar.sqrt(lv, l2)
                xx0 = t_new("xx0")
                tt(xx0, x, lv, OP.mult)         # X = x*L
                # +eps: degenerate dihedrals (repeated atoms -> H=0) resolve
                # to cos(theta)=1, matching the reference's atan2(0,0)=0.
                xx = t_new("xx")
                nc.vector.tensor_scalar_add(xx, xx0, 1e-15)

                x2 = t_new("x2")
                tt(x2, xx, xx, OP.mult)
                u2t = t_new("u2t")
                tt(u2t, u, u, OP.mult)
                h2 = t_new("h2")
                tt(h2, x2, u2t, OP.add)
                rh2 = t_new("rh2")
                nc.vector.reciprocal(rh2, h2)    # 1/H^2
                rh = t_new("rh")
                nc.scalar.sqrt(rh, rh2)          # 1/H

                # P1 = B*X + C*u
                p1a = t_new("p1a")
                tt(p1a, pv[:, :, 1], xx, OP.mult)
                p1b = t_new("p1b")
                tt(p1b, pv[:, :, 2], u, OP.mult)
                p1 = t_new("p1")
                tt(p1, p1a, p1b, OP.add)
                # Q = D*(X^2-u^2) + E*X*u
                xmu = t_new("xmu")
                tt(xmu, x2, u2t, OP.subtract)
                q1 = t_new("q1")
                tt(q1, pv[:, :, 3], xmu, OP.mult)
                xu = t_new("xu")
                tt(xu, xx, u, OP.mult)
                q2 = t_new("q2")
                tt(q2, pv[:, :, 4], xu, OP.mult)
                qq = t_new("qq")
                tt(qq, q1, q2, OP.add)
                # V = A - P1/H - Q/H^2
                m1t = t_new("m1t")
                tt(m1t, p1, rh, OP.mult)
                m2t = t_new("m2t")
                tt(m2t, qq, rh2, OP.mult)
                v1 = t_new("v1")
                tt(v1, pv[:, :, 0], m1t, OP.subtract)
                v2 = t_new("v2")
                tt(v2, v1, m2t, OP.subtract)
                # reference NaNs exactly when |b2|==0 (its 0/0 normalize);
                # 0*recip(L^2) is 0 normally, NaN when L^2==0.
                rl2 = t_new("rl2")
                nc.vector.reciprocal(rl2, l2)
                nanz = t_new("nanz")
                nc.vector.tensor_scalar_mul(nanz, rl2, 0.0)
                vt = pool.tile([P, UNROLL], F32, tag="vt", name="vt")
                tt(vt, v2, nanz, OP.add)

                nc.sync.dma_start(
                    out=vout[bass.ds(it, CHUNK)].rearrange("(p j) -> p j", p=P),
                    in_=vt[:],
                )

    nc.finalize()
    return nc


_NC_CACHE = None


def _get_nc():
    global _NC_CACHE
    if _NC_CACHE is None:
        _NC_CACHE = _build_nc()
    return _NC_CACHE


def kernel(pos, theta_0, k_0, theta_1, k_1, mapping, atom_types):
    pos = np.asarray(pos, dtype=np.float32)
    mapping = np.asarray(mapping)
    atom_types = np.asarray(atom_types)

    # pack parameter tables -> [26^4, 5] f32 rows [A, B, C, D, E]
    a0 = np.asarray(theta_0, np.float32).reshape(-1)
    a1 = np.asarray(theta_1, np.float32).reshape(-1)
    K0 = np.asarray(k_0, np.float32).reshape(-1)
    K1 = np.asarray(k_1, np.float32).reshape(-1)
    prm = np.zeros((N_TYPES ** 4, 5), np.float32)
    prm[:, 0] = K0 + K1
    prm[:, 1] = K0 * np.cos(a0)
    prm[:, 2] = -K0 * np.sin(a0)
    prm[:, 3] = K1 * np.cos(a1)
    prm[:, 4] = -2.0 * K1 * np.sin(a1)

    # atom ids fit in 21 bits; stash the atom's type in bits 21+ so the
    # device can unpack both from one int32 (saves uploading a type lane)
    ty32 = atom_types.astype(np.int32)
    mp32 = np.ascontiguousarray(
        mapping.astype(np.int32) | (ty32[mapping] << 21)
    )
    pos = np.ascontiguousarray(pos)

    nc = _get_nc()
    in_maps = []
    for c in range(N_CORES):
        sl = mp32[:, c * MC : (c + 1) * MC]
        mp_pad = np.empty((4, MC_PAD), np.int32)
        mp_pad[:, :MC] = sl
        # padding uses distinct atoms so no 0/0 NaNs are generated
        pad_ids = np.arange(4, dtype=np.int32)
        mp_pad[:, MC:] = (pad_ids | (ty32[pad_ids] << 21))[:, None]
        in_maps.append({"pos3": pos, "prm": prm, "mp": mp_pad})

    res = run_bass_kernel_spmd(nc, in_maps, core_ids=list(range(N_CORES)))
    out = np.empty(N_DIH, np.float32)
    for c in range(N_CORES):
        out[c * MC : (c + 1) * MC] = res.results[c]["V"][:MC]
    return out
